# revision 1
# baseline (speedup 1.0000x reference)
"""BurstGNN Trainium2 kernel (8 NeuronCores, SPMD, 3 launches).

Sharding: nodes/edges partitioned by dst across 8 cores (graph partitioning
per the hint); small weights replicated. Edges are sorted into 32-node dst
windows (8 windows -> one 256-node group), padded to a uniform number of
128-edge subtiles per window so one Bass program serves all cores.

The host resolves all indices: it pre-gathers the per-edge source-row streams
(h[src], al[src]) between launches (halo exchange done host-side), computes
the gcn norm, and lays out window-local scatter metadata. The device performs
the FAConv compute: alpha = tanh(al_src + ar_dst) * norm via one-hot
compare + reduce for the ar expansion, and the scatter-sum via selection-
matrix matmuls accumulating in PSUM, plus the ragged per-user segment sums
and the final MLP.

Launch A: layer-1 FAConv -> x1 slices (+ al2/ar2).
Launch B: layer-2 FAConv -> pre-smoothing x2 slices.
Launch C: per-user segment sums (re_index resolved) + final MLP -> logits.
"""

import sys

sys.path.insert(0, "/opt/trn_rl_repo")

import ml_dtypes
import numpy as np

import concourse.bass as bass
import concourse.bacc as bacc
import concourse.mybir as mybir
import concourse.tile as tile

F32 = mybir.dt.float32
BF16 = mybir.dt.bfloat16
AF = mybir.ActivationFunctionType
OP = mybir.AluOpType

EPS = 0.1
LRELU_SLOPE = 0.01


class Cfg:
    def __init__(self, N=200000, E=1600000, U=20000, NUMP=20, CATP=12):
        self.N, self.E, self.U = N, E, U
        self.NUMP, self.CATP = NUMP, CATP
        self.C = 8
        self.D = 64
        self.WJ = 32
        self.GRP = 16
        self.NS = N // self.C
        span = self.WJ * self.GRP
        self.NSP = ((self.NS + span - 1) // span) * span
        self.W = self.NSP // self.WJ
        self.G = self.W // self.GRP
        self.TBLR = self.C * self.NSP
        self.UPCU = U // self.C
        self.UW = (self.UPCU + 127) // 128
        self.UPC = self.UW * 128


def _mkap(handle, offset, dims):
    return bass.AP(handle, int(offset), [list(d) for d in dims])


def _fap(base, dims, extra_off=0):
    return bass.AP(base.tensor, base.offset + extra_off,
                   [list(base.ap[0])] + [list(d) for d in dims])


# --------------------------------------------------------------------------
# Host preprocessing
# --------------------------------------------------------------------------

def preprocess(inputs, cfg):
    """Edge/user slotting + host encoder. Returns slot metadata and host
    arrays needed to build per-launch inputs."""
    c = cfg
    src = np.asarray(inputs["edge_index"][0], dtype=np.int64)
    dst = np.asarray(inputs["edge_index"][1], dtype=np.int64)
    offs = np.asarray(inputs["tweet_offsets"], dtype=np.int64)
    re_index = np.asarray(inputs["re_index"], dtype=np.int64)

    deg = np.bincount(dst, minlength=c.N).astype(np.float64) + 1.0
    dinv = (deg ** -0.5).astype(np.float32)

    srcA = np.concatenate([src, np.arange(c.N, dtype=np.int64)])
    dstA = np.concatenate([dst, np.arange(c.N, dtype=np.int64)])
    normA = dinv[srcA] * dinv[dstA]

    core = dstA // c.NS
    dl = dstA - core * c.NS
    wloc = dl // c.WJ
    jloc = (dl - wloc * c.WJ).astype(np.float32)
    gwin = core * c.W + wloc

    cnt = np.bincount(gwin, minlength=c.C * c.W)
    T = max(1, int(-(-cnt.max() // 128)))
    K = c.GRP * T

    order = np.argsort(gwin, kind="stable")
    starts = np.zeros(c.C * c.W + 1, np.int64)
    np.cumsum(cnt, out=starts[1:])
    ranks = np.arange(len(gwin), dtype=np.int64) - starts[gwin[order]]
    t_ = ranks // 128
    p_ = ranks - t_ * 128
    wo = wloc[order]
    k_ = (wo % c.GRP) * T + t_
    g_ = wo // c.GRP
    co = core[order]
    flat = (g_ * 128 + p_) * K + k_

    sz = c.G * 128 * K
    meta_dl = np.full((c.C, sz), -1.0, np.float32)
    slot_norm = np.zeros((c.C, sz), np.float32)
    # global (padded) rows per slot; pads -> row TBLR-1 (zeros)
    slot_row = np.full((c.C, sz), c.TBLR - 1, np.int64)
    slot_drow = np.full((c.C, sz), c.TBLR - 1, np.int64)
    rowidx = (srcA // c.NS) * c.NSP + (srcA % c.NS)
    drowidx = core * c.NSP + dl
    meta_dl[co, flat] = jloc[order]
    slot_norm[co, flat] = normA[order]
    slot_row[co, flat] = rowidx[order]
    slot_drow[co, flat] = drowidx[order]

    meta_f = np.ascontiguousarray(meta_dl.reshape(c.C, c.G, 128, K))

    # ---- user phase ----
    st = offs[re_index]
    ln = (offs[re_index + 1] - st).astype(np.int64)
    tot = int(ln.sum())
    uu = np.repeat(np.arange(c.U, dtype=np.int64), ln)
    csl = np.cumsum(ln) - ln
    pos = np.arange(tot, dtype=np.int64) - np.repeat(csl, ln)
    nodes = np.repeat(st, ln) + pos
    ucore = uu // c.UPCU
    ulocal = uu - ucore * c.UPCU
    uw = ulocal // 128
    uj = (ulocal - uw * 128).astype(np.float32)
    guw = ucore * c.UW + uw
    ucnt = np.bincount(guw, minlength=c.C * c.UW)
    KU = max(1, int(-(-ucnt.max() // 128)))

    ustarts = np.zeros(c.C * c.UW + 1, np.int64)
    np.cumsum(ucnt, out=ustarts[1:])
    uranks = np.arange(tot, dtype=np.int64) - ustarts[guw]
    ut = uranks // 128
    up = uranks - ut * 128
    uflat = (uw * 128 + up) * KU + ut

    usz = c.UW * 128 * KU
    umeta_j = np.full((c.C, usz), -1.0, np.float32)
    uslot_row = np.full((c.C, usz), c.TBLR - 1, np.int64)
    urowidx = (nodes // c.NS) * c.NSP + (nodes % c.NS)
    umeta_j[ucore, uflat] = uj
    uslot_row[ucore, uflat] = urowidx
    umeta_f = umeta_j.reshape(c.C, c.UW, 128, KU)

    # ---- host encoder (cheap: N x 96 x 64 flops) ----
    lrelu = lambda v: np.where(v > 0, v, np.float32(LRELU_SLOPE) * v).astype(np.float32)
    num = lrelu(np.asarray(inputs["num_prop"], np.float32) @
                np.asarray(inputs["W_num"], np.float32) +
                np.asarray(inputs["b_num"], np.float32))
    cat = lrelu(np.asarray(inputs["cat_prop"], np.float32) @
                np.asarray(inputs["W_cat"], np.float32) +
                np.asarray(inputs["b_cat"], np.float32))
    x = lrelu(np.concatenate([num, cat], axis=1) @
              np.asarray(inputs["W_tog"], np.float32) +
              np.asarray(inputs["b_tog"], np.float32))
    att_l = np.asarray(inputs["att_l"], np.float32)
    att_r = np.asarray(inputs["att_r"], np.float32)

    # padded-global layout [TBLR, 66]: cols x(64), al, ar; pad rows zero
    fullx = np.zeros((c.TBLR, 66), np.float32)
    for cc in range(c.C):
        r0 = cc * c.NSP
        fullx[r0:r0 + c.NS, :64] = x[cc * c.NS:(cc + 1) * c.NS]
    fullx[:, 64] = fullx[:, :64] @ att_l
    fullx[:, 65] = fullx[:, :64] @ att_r

    meta = dict(T=T, K=K, KU=KU, meta_f=meta_f, slot_row=slot_row,
                slot_drow=slot_drow, slot_norm=slot_norm,
                umeta_f=umeta_f, uslot_row=uslot_row, fullx=fullx,
                att_l=att_l, att_r=att_r)
    return meta


def gather_stream(cfg, meta, table):
    """Host halo-exchange: per-core per-slot x(64) stream (bf16)."""
    c = cfg
    out = table[:, :64][meta["slot_row"].reshape(c.C, -1)]
    out = out.astype(ml_dtypes.bfloat16)
    return np.ascontiguousarray(out.reshape(c.C, c.G, 128, meta["K"] * 64))


def alpha_slots(cfg, meta, al, ar):
    """Host per-slot [dst_local | alpha] metadata for one launch."""
    c = cfg
    K = meta["K"]
    a = np.tanh(al[meta["slot_row"]] + ar[meta["slot_drow"]],
                dtype=np.float32) * meta["slot_norm"]
    both = np.concatenate([meta["meta_f"],
                           a.astype(np.float32).reshape(c.C, c.G, 128, K)],
                          axis=3)
    return np.ascontiguousarray(both)


# --------------------------------------------------------------------------
# Bass programs
# --------------------------------------------------------------------------

def build_layer_program(cfg, T):
    """FAConv layer: x[src] stream + dst_local meta + host alpha + x0 -> out."""
    c = cfg
    K = c.GRP * T
    nc = bacc.Bacc()
    stream = nc.declare_dram_parameter("stream", [c.G, 128, K * 64], BF16,
                                       isOutput=False)
    alpha_p = nc.declare_dram_parameter("alpha", [c.G, 128, 2 * K], F32,
                                        isOutput=False)
    x0_p = nc.declare_dram_parameter("x0", [c.NSP, c.D], F32, isOutput=False)
    iota32 = nc.declare_dram_parameter("iota32", [128, 32], F32, isOutput=False)
    out_p = nc.declare_dram_parameter("out", [c.NSP, 64], F32, isOutput=True)

    with tile.TileContext(nc) as tc:
        with tc.tile_pool(name="consts", bufs=1) as cp:
            iota32_s = cp.tile([128, 32], F32)
            nc.sync.dma_start(out=iota32_s[:], in_=iota32[:, :])
            la_tiles = []
            for j in range(2):
                lt = cp.tile([128, c.GRP * T, 128], BF16, tag=f"la{j}")
                nc.vector.memset(lt[:].rearrange("p k f -> p (k f)"), 0.0)
                la_tiles.append(lt)

            with tc.tile_pool(name="lay", bufs=6) as lp, \
                 tc.tile_pool(name="layps", bufs=6, space="PSUM") as pp:
                for g in range(c.G):
                    mfa = lp.tile([128, 2 * K], F32, tag="mfa")
                    nc.sync.dma_start(out=mfa[:], in_=alpha_p[g])
                    mf = mfa[:, 0:K]
                    alp = mfa[:, K:2 * K]
                    hg = lp.tile([128, K, 64], BF16, tag="hg")
                    nc.sync.dma_start(
                        out=hg[:].rearrange("p k f -> p (k f)"), in_=stream[g])
                    m01 = lp.tile([128, K, 32], F32, tag="m01")
                    nc.vector.tensor_tensor(
                        out=m01[:],
                        in0=_fap(mf, [[1, K], [0, 32]]),
                        in1=_fap(iota32_s[:], [[0, K], [1, 32]]),
                        op=OP.is_equal)
                    la = la_tiles[g % 2]
                    for half in range(c.GRP // 4):
                        h4t = half * 4 * T
                        nc.vector.tensor_tensor(
                            out=_fap(la[:], [[T * 128 + 32, 4], [128, T],
                                             [1, 32]], extra_off=h4t * 128),
                            in0=m01[:, h4t:h4t + 4 * T, :],
                            in1=_fap(alp, [[1, 4 * T], [0, 32]],
                                     extra_off=h4t),
                            op=OP.mult)
                        ps = pp.tile([128, 64], F32, tag="agg")
                        for kk in range(4 * T):
                            k = h4t + kk
                            nc.tensor.matmul(
                                out=ps[:], lhsT=la[:, k, :],
                                rhs=hg[:, k, :],
                                start=(kk == 0), stop=(kk == 4 * T - 1))
                        base = g * 32 * c.GRP + half * 128
                        x0b = lp.tile([128, 64], F32, tag="x0b")
                        nc.scalar.dma_start(out=x0b[:],
                                            in_=x0_p[base:base + 128, :])
                        xo = lp.tile([128, 64], F32, tag="xo")
                        nc.vector.scalar_tensor_tensor(
                            out=xo[:], in0=x0b[:], scalar=EPS,
                            in1=ps[:], op0=OP.mult, op1=OP.add)
                        nc.sync.dma_start(out=out_p[base:base + 128, :],
                                          in_=xo[:])
    nc.finalize()
    return nc


def build_user_program(cfg, KU):
    c = cfg
    nc = bacc.Bacc()
    ustream = nc.declare_dram_parameter("ustream", [c.UW, 128, KU * 64], F32,
                                        isOutput=False)
    umeta_f = nc.declare_dram_parameter("umeta_f", [c.UW, 128, KU], F32,
                                        isOutput=False)
    w_f1 = nc.declare_dram_parameter("w_f1", [64, 32], F32, isOutput=False)
    b_f1c = nc.declare_dram_parameter("b_f1c", [32, 1], F32, isOutput=False)
    w_lab = nc.declare_dram_parameter("w_lab", [32, 2], F32, isOutput=False)
    b_labc = nc.declare_dram_parameter("b_labc", [2, 1], F32, isOutput=False)
    iota128 = nc.declare_dram_parameter("iota128", [128, 128], F32, isOutput=False)
    ident = nc.declare_dram_parameter("ident", [128, 128], F32, isOutput=False)
    out_p = nc.declare_dram_parameter("out", [2, c.UPC], F32, isOutput=True)

    with tile.TileContext(nc) as tc:
        with tc.tile_pool(name="consts", bufs=1) as cp:
            wf1_s = cp.tile([64, 32], F32)
            nc.sync.dma_start(out=wf1_s[:], in_=w_f1[:, :])
            bf1_s = cp.tile([32, 1], F32)
            nc.sync.dma_start(out=bf1_s[:], in_=b_f1c[:, :])
            wlab_s = cp.tile([32, 2], F32)
            nc.sync.dma_start(out=wlab_s[:], in_=w_lab[:, :])
            blab_s = cp.tile([2, 1], F32)
            nc.sync.dma_start(out=blab_s[:], in_=b_labc[:, :])
            iota128_s = cp.tile([128, 128], F32)
            nc.sync.dma_start(out=iota128_s[:], in_=iota128[:, :])
            ident_s = cp.tile([128, 128], F32)
            nc.sync.dma_start(out=ident_s[:], in_=ident[:, :])

            with tc.tile_pool(name="usr", bufs=3) as up, \
                 tc.tile_pool(name="usrps", bufs=2, space="PSUM") as ups:
                for uw in range(c.UW):
                    umf = up.tile([128, KU], F32, tag="umf")
                    nc.sync.dma_start(out=umf[:], in_=umeta_f[uw])
                    ug = up.tile([128, KU, 64], F32, tag="ug")
                    nc.sync.dma_start(
                        out=ug[:].rearrange("p k f -> p (k f)"),
                        in_=ustream[uw])
                    m01u = up.tile([128, KU, 128], F32, tag="m01u")
                    nc.vector.tensor_tensor(
                        out=m01u[:],
                        in0=_fap(umf[:], [[1, KU], [0, 128]]),
                        in1=_fap(iota128_s[:], [[0, KU], [1, 128]]),
                        op=OP.is_equal)
                    psy = ups.tile([128, 64], F32, tag="psy")
                    for k in range(KU):
                        nc.tensor.matmul(out=psy[:], lhsT=m01u[:, k, :],
                                         rhs=ug[:, k, :],
                                         start=(k == 0), stop=(k == KU - 1))
                    ys = up.tile([128, 64], F32, tag="ys")
                    nc.scalar.copy(out=ys[:], in_=psy[:])
                    ytp = ups.tile([64, 128], F32, tag="ytp")
                    nc.tensor.transpose(out=ytp[:], in_=ys[:],
                                        identity=ident_s[:])
                    yts = up.tile([64, 128], F32, tag="yts")
                    nc.scalar.copy(out=yts[:], in_=ytp[:])
                    h1p = ups.tile([32, 128], F32, tag="h1p")
                    nc.tensor.matmul(out=h1p[:], lhsT=wf1_s[:], rhs=yts[:],
                                     start=True, stop=True)
                    h1b = up.tile([32, 128], F32, tag="h1b")
                    nc.scalar.activation(out=h1b[:], in_=h1p[:],
                                         func=AF.Identity, bias=bf1_s[:, 0:1])
                    h1s = up.tile([32, 128], F32, tag="h1s")
                    nc.vector.scalar_tensor_tensor(
                        out=h1s[:], in0=h1b[:], scalar=LRELU_SLOPE,
                        in1=h1b[:], op0=OP.mult, op1=OP.max)
                    o2p = ups.tile([2, 128], F32, tag="o2p")
                    nc.tensor.matmul(out=o2p[:], lhsT=wlab_s[:], rhs=h1s[:],
                                     start=True, stop=True)
                    o2s = up.tile([2, 128], F32, tag="o2s")
                    nc.scalar.activation(out=o2s[:], in_=o2p[:],
                                         func=AF.Identity, bias=blab_s[:, 0:1])
                    nc.sync.dma_start(out=out_p[:, 128 * uw:128 * (uw + 1)],
                                      in_=o2s[:])
    nc.finalize()
    return nc


# --------------------------------------------------------------------------
# Entry point
# --------------------------------------------------------------------------

_CACHE = {}


def _prog(key, builder, *args):
    if key not in _CACHE:
        _CACHE[key] = builder(*args)
    return _CACHE[key]


def run_all(inputs, cfg, runner):
    """runner(nc, in_maps) -> list of per-core output dicts."""
    c = cfg
    meta = preprocess(inputs, cfg)
    T, KU = meta["T"], meta["KU"]
    fullx = meta["fullx"]
    att_l, att_r = meta["att_l"], meta["att_r"]

    iota32 = np.tile(np.arange(32, dtype=np.float32)[None, :], (128, 1))
    iota128 = np.tile(np.arange(128, dtype=np.float32)[None, :], (128, 1))
    ident = np.eye(128, dtype=np.float32)

    x0 = np.ascontiguousarray(fullx[:, :64].reshape(c.C, c.NSP, 64))
    ncL = _prog(("lay", c.N, T), build_layer_program, cfg, T)

    # ---- launch A: layer 1 (host alpha from encoder al/ar) ----
    s1 = gather_stream(cfg, meta, fullx)
    a1 = alpha_slots(cfg, meta, fullx[:, 64], fullx[:, 65])
    mapsA = [{"stream": s1[cc], "alpha": a1[cc], "x0": x0[cc],
              "iota32": iota32} for cc in range(c.C)]
    resA = runner(ncL, mapsA)

    fullx1 = np.zeros((c.TBLR, 66), np.float32)
    for cc in range(c.C):
        fullx1[cc * c.NSP:(cc + 1) * c.NSP, :64] = resA[cc]["out"]
    fullx1[:, 64] = fullx1[:, :64] @ att_l
    fullx1[:, 65] = fullx1[:, :64] @ att_r

    # ---- launch B: layer 2 ----
    s2 = gather_stream(cfg, meta, fullx1)
    a2 = alpha_slots(cfg, meta, fullx1[:, 64], fullx1[:, 65])
    mapsB = [{"stream": s2[cc], "alpha": a2[cc], "x0": x0[cc],
              "iota32": iota32} for cc in range(c.C)]
    resB = runner(ncL, mapsB)

    fullx2 = np.zeros((c.TBLR, 64), np.float32)
    for cc in range(c.C):
        fullx2[cc * c.NSP:(cc + 1) * c.NSP] = resB[cc]["out"]
    # exact smoothing (reference: sqrt(x^2 + 1e-8)), elementwise on host
    fullx2 = np.sqrt(fullx2 * fullx2 + np.float32(1e-8), dtype=np.float32)
    # keep pad rows zero so padded user slots contribute nothing
    pad = np.ones(c.TBLR, bool)
    for cc in range(c.C):
        pad[cc * c.NSP:cc * c.NSP + c.NS] = False
    fullx2[pad] = 0.0

    # ---- launch C: user segment sums + MLP ----
    us = fullx2[meta["uslot_row"].reshape(c.C, -1)].reshape(
        c.C, c.UW, 128, KU * 64)
    ncC = _prog(("usr", c.N, c.U, KU), build_user_program, cfg, KU)
    mapsC = [{"ustream": np.ascontiguousarray(us[cc]),
              "umeta_f": meta["umeta_f"][cc],
              "w_f1": np.asarray(inputs["W_f1"], np.float32),
              "b_f1c": np.ascontiguousarray(
                  np.asarray(inputs["b_f1"], np.float32).reshape(32, 1)),
              "w_lab": np.asarray(inputs["W_lab"], np.float32),
              "b_labc": np.ascontiguousarray(
                  np.asarray(inputs["b_lab"], np.float32).reshape(2, 1)),
              "iota128": iota128, "ident": ident} for cc in range(c.C)]
    resC = runner(ncC, mapsC)

    out = np.zeros((c.U, 2), np.float32)
    for cc in range(c.C):
        out[cc * c.UPCU:(cc + 1) * c.UPCU, :] = \
            resC[cc]["out"][:, :c.UPCU].T
    return out


def kernel(**inputs):
    from concourse.bass_utils import run_bass_kernel_spmd
    cfg = Cfg()

    def runner(nc, in_maps):
        return run_bass_kernel_spmd(nc, in_maps,
                                    core_ids=list(range(cfg.C))).results

    return run_all(inputs, cfg, runner)



# revision 8
# speedup vs baseline: 9.1866x; 9.1866x over previous
"""BurstGNN Trainium2 kernel — single fused launch on 8 NeuronCores (SPMD).

Sharding: nodes/edges partitioned by dst across the 8 cores (graph
partitioning per the hint); small weights replicated; the FAConv halo
exchange is a device-side AllGather of the per-core node-table shard.

Device program (one Bass program, one launch):
  1. Encoder: propsT shard -> x0 (leaky-relu MLP), al/ar attention dots,
     writes the extended node-table shard [x*dinv | al].
  2. AllGather shard -> full node table (halo exchange).
  3. FAConv layer 1: per 128-dst block, indirect-DMA gather of source rows,
     alpha = tanh(al_src + ar_dst) built with a one-hot compare against the
     block-local dst offset, scatter-add via selection-matrix matmuls in
     PSUM; writes the next extended table shard.
  4. AllGather again; FAConv layer 2 + smoothing sqrt(x^2+1e-8).
  5. Per-user segment sums (users are contiguous row ranges; each core
     owns a contiguous user window) with the same one-hot machinery.
Host: builds slot metadata (numpy), applies re_index + the tiny final MLP.

Everything is keyed off the fixed problem shapes below; K_E/K_U (slot
columns per 128-row block) are derived from the input data at runtime.
"""

import os
import sys

sys.path.insert(0, "/opt/trn_rl_repo")

import numpy as np

# Persistent XLA compile cache: makes recompiles of the identical program
# (same data -> same BIR) a cache hit across processes.
try:
    import jax
    os.makedirs("/root/jaxcache", exist_ok=True)
    jax.config.update("jax_compilation_cache_dir", "/root/jaxcache")
    jax.config.update("jax_persistent_cache_min_entry_size_bytes", -1)
    jax.config.update("jax_persistent_cache_min_compile_time_secs", 0.0)
except Exception:
    pass

import concourse.bass as bass
import concourse.bacc as bacc
import concourse.mybir as mybir
import concourse.tile as tile

F32 = mybir.dt.float32
I32 = mybir.dt.int32
AF = mybir.ActivationFunctionType
OP = mybir.AluOpType

# problem shapes
N, E, U = 200000, 1600000, 20000
NUMP, CATP, D = 20, 12, 64
C = 8
NS = N // C            # 25000 rows per core
NB = (NS + 127) // 128  # 196 dst blocks per core
NSP = NB * 128          # 25088 padded rows per core
TBLR = C * NSP          # 200704 global padded rows
ROWB = 68               # table row floats: x*dinv (64) | al | 3 pad
UHB = 24                # user blocks per core
UH = UHB * 128          # 3072 user slots per core window
EPS = 0.1
SLOPE = 0.01


def _v(t, dims, off=0):
    """View of a tile AP with custom free dims (keeps partition dim)."""
    return bass.AP(t.tensor, t.offset + off,
                   [list(t.ap[0])] + [list(d) for d in dims])


def _dap(handle, off, dims):
    return bass.AP(handle, int(off), [list(d) for d in dims])


# --------------------------------------------------------------------------
# Host preprocessing
# --------------------------------------------------------------------------

def preprocess(inputs):
    src = np.asarray(inputs["edge_index"][0]).astype(np.int64, copy=False)
    dst = np.asarray(inputs["edge_index"][1]).astype(np.int64, copy=False)
    offs = np.asarray(inputs["tweet_offsets"]).astype(np.int64, copy=False)

    loop = np.arange(N, dtype=np.int64)
    srcA = np.concatenate([src, loop])
    dstA = np.concatenate([dst, loop])
    M = srcA.shape[0]

    deg = np.bincount(dstA, minlength=N).astype(np.float64)
    dinv = np.where(deg > 0, deg ** -0.5, 0.0).astype(np.float32)

    core = dstA // NS
    dloc = dstA - core * NS
    blk = dloc >> 7
    f_off = (dloc & 127).astype(np.float32)
    gblk = core * NB + blk

    cnt = np.bincount(gblk, minlength=C * NB)
    # per-block columns = max over cores so the SPMD program is uniform
    cnt2 = cnt.reshape(C, NB)
    kb_e = np.maximum(1, -(-cnt2.max(axis=0) // 128)).astype(np.int64)  # [NB]
    KE = int(kb_e.max())

    order = np.argsort(gblk, kind="stable")
    starts = np.zeros(C * NB + 1, np.int64)
    np.cumsum(cnt, out=starts[1:])
    ranks = np.arange(M, dtype=np.int64) - starts[gblk[order]]
    kk = ranks >> 7
    pp = ranks & 127
    gpos = gblk[order] * (128 * KE) + pp * KE + kk

    idx_e = np.zeros(C * NB * 128 * KE, np.int32)
    mf_e = np.full(C * NB * 128 * KE, -1.0, np.float32)
    so = srcA[order]
    sc = so // NS
    idx_e[gpos] = (sc * NSP + (so - sc * NS)).astype(np.int32)
    mf_e[gpos] = f_off[order]
    idx_e = idx_e.reshape(C, NB, 128, KE)
    mf_e = mf_e.reshape(C, NB, 128, KE)

    # ---- user phase ----
    seg = np.searchsorted(offs, np.arange(N, dtype=np.int64),
                          side="right") - 1
    seg = np.clip(seg, 0, U - 1)
    core_n = np.arange(N, dtype=np.int64) // NS
    ulo = seg[np.arange(C, dtype=np.int64) * NS]
    ul = seg - ulo[core_n]
    assert ul.min() >= 0 and ul.max() < UH, \
        f"user window overflow: {ul.max()}"
    ublk = ul >> 7
    uoff = (ul & 127).astype(np.float32)
    gublk = core_n * UHB + ublk
    ucnt = np.bincount(gublk, minlength=C * UHB)
    ucnt2 = ucnt.reshape(C, UHB)
    kb_u = np.maximum(1, -(-ucnt2.max(axis=0) // 128)).astype(np.int64)
    KU = int(kb_u.max())

    change = np.empty(N, bool)
    change[0] = True
    np.not_equal(gublk[1:], gublk[:-1], out=change[1:])
    run_id = np.cumsum(change) - 1
    run_start = np.flatnonzero(change)
    ranks_u = np.arange(N, dtype=np.int64) - run_start[run_id]
    ku = ranks_u >> 7
    pu = ranks_u & 127
    gposu = gublk * (128 * KU) + pu * KU + ku

    idx_u = np.zeros(C * UHB * 128 * KU, np.int32)
    mf_u = np.full(C * UHB * 128 * KU, -1.0, np.float32)
    idx_u[gposu] = (np.arange(N, dtype=np.int64) - core_n * NS).astype(np.int32)
    mf_u[gposu] = uoff
    idx_u = idx_u.reshape(C, UHB, 128, KU)
    mf_u = mf_u.reshape(C, UHB, 128, KU)

    # ---- per-core dense inputs ----
    num = np.asarray(inputs["num_prop"], np.float32)
    cat = np.asarray(inputs["cat_prop"], np.float32)
    propsT = np.zeros((C, 44, NSP), np.float32)
    dinvt = np.zeros((C, 128, NB), np.float32)
    for c in range(C):
        sl = slice(c * NS, (c + 1) * NS)
        propsT[c, 0:NUMP, 0:NS] = num[sl].T
        propsT[c, 32:32 + CATP, 0:NS] = cat[sl].T
        dl = np.zeros(NSP, np.float32)
        dl[:NS] = dinv[sl]
        dinvt[c] = dl.reshape(NB, 128).T

    return dict(KE=KE, KU=KU, kb_e=kb_e, kb_u=kb_u,
                idx_e=idx_e, mf_e=mf_e, idx_u=idx_u, mf_u=mf_u,
                propsT=propsT, dinvt=dinvt, ulo=ulo)


# --------------------------------------------------------------------------
# Device program
# --------------------------------------------------------------------------

def build_program(KE, KU, kb_e, kb_u):
    kb_e = [int(x) for x in kb_e]
    kb_u = [int(x) for x in kb_u]
    nc = bacc.Bacc()

    propsT = nc.declare_dram_parameter("propsT", [44, NSP], F32, isOutput=False)
    dinvt = nc.declare_dram_parameter("dinvt", [128, NB], F32, isOutput=False)
    wnum = nc.declare_dram_parameter("wnum", [NUMP, 32], F32, isOutput=False)
    wcat = nc.declare_dram_parameter("wcat", [CATP, 32], F32, isOutput=False)
    wtog = nc.declare_dram_parameter("wtog", [64, 64], F32, isOutput=False)
    bnum = nc.declare_dram_parameter("bnum", [32, 1], F32, isOutput=False)
    bcat = nc.declare_dram_parameter("bcat", [32, 1], F32, isOutput=False)
    btog = nc.declare_dram_parameter("btog", [64, 1], F32, isOutput=False)
    btogr = nc.declare_dram_parameter("btogr", [1, 64], F32, isOutput=False)
    attp = nc.declare_dram_parameter("attp", [64, 2], F32, isOutput=False)
    iota = nc.declare_dram_parameter("iota", [128, 128], F32, isOutput=False)
    ident = nc.declare_dram_parameter("ident", [128, 128], F32, isOutput=False)
    idx_e = nc.declare_dram_parameter("idx_e", [NB, 128, KE], I32, isOutput=False)
    mf_e = nc.declare_dram_parameter("mf_e", [NB, 128, KE], F32, isOutput=False)
    idx_u = nc.declare_dram_parameter("idx_u", [UHB, 128, KU], I32, isOutput=False)
    mf_u = nc.declare_dram_parameter("mf_u", [UHB, 128, KU], F32, isOutput=False)
    usums = nc.declare_dram_parameter("usums", [UH, 64], F32, isOutput=True)

    shard0 = nc.dram_tensor("shard0", [NSP, ROWB], F32, kind="Internal")
    shard1 = nc.dram_tensor("shard1", [NSP, ROWB], F32, kind="Internal")
    xg0 = nc.dram_tensor("xg0", [TBLR, ROWB], F32, kind="Internal")
    xg1 = nc.dram_tensor("xg1", [TBLR, ROWB], F32, kind="Internal")
    arr0 = nc.dram_tensor("arr0", [NSP, 1], F32, kind="Internal")
    arr1 = nc.dram_tensor("arr1", [NSP, 1], F32, kind="Internal")
    x0loc = nc.dram_tensor("x0loc", [NSP, 64], F32, kind="Internal")
    x2sloc = nc.dram_tensor("x2sloc", [NSP, 64], F32, kind="Internal")

    with tile.TileContext(nc) as tc:
        with tc.tile_pool(name="consts", bufs=1) as cp:
            wnum_s = cp.tile([NUMP, 32], F32)
            nc.sync.dma_start(out=wnum_s[:], in_=wnum[:, :])
            wcat_s = cp.tile([32 + CATP, 32], F32)
            nc.sync.dma_start(out=wcat_s[32:32 + CATP, :], in_=wcat[:, :])
            wtog_s = cp.tile([64, 64], F32)
            nc.sync.dma_start(out=wtog_s[:], in_=wtog[:, :])
            bnum_s = cp.tile([32, 1], F32)
            nc.sync.dma_start(out=bnum_s[:], in_=bnum[:, :])
            bcat_s = cp.tile([32, 1], F32)
            nc.sync.dma_start(out=bcat_s[:], in_=bcat[:, :])
            btog_s = cp.tile([64, 1], F32)
            nc.sync.dma_start(out=btog_s[:], in_=btog[:, :])
            btog_b = cp.tile([128, 64], F32)
            nc.sync.dma_start(out=btog_b[:],
                              in_=_dap(btogr, 0, [[0, 128], [1, 64]]))
            attp_s = cp.tile([64, 2], F32)
            nc.sync.dma_start(out=attp_s[:], in_=attp[:, :])
            iota_s = cp.tile([128, 128], F32)
            nc.sync.dma_start(out=iota_s[:], in_=iota[:, :])
            ident_s = cp.tile([128, 128], F32)
            nc.sync.dma_start(out=ident_s[:], in_=ident[:, :])
            dinv_s = cp.tile([128, NB], F32)
            nc.sync.dma_start(out=dinv_s[:], in_=dinvt[:, :])
            eps_s = cp.tile([128, 1], F32)
            nc.vector.memset(eps_s[:], 1e-8)

            # ---------------- encoder ----------------
            with tc.tile_pool(name="enc", bufs=3) as ep, \
                 tc.tile_pool(name="encps", bufs=2, space="PSUM") as epp:
                for i in range(NB // 4):
                    r0 = i * 512
                    pT = ep.tile([44, 512], F32, tag="pT")
                    nc.sync.dma_start(out=pT[:], in_=propsT[:, r0:r0 + 512])
                    psH = epp.tile([64, 512], F32, tag="psH")
                    nc.tensor.matmul(out=psH[0:32, :], lhsT=wnum_s[:],
                                     rhs=pT[0:NUMP, :], start=True, stop=True)
                    nc.tensor.matmul(out=psH[32:64, :],
                                     lhsT=wcat_s[32:32 + CATP, :],
                                     rhs=pT[32:32 + CATP, :],
                                     start=True, stop=True)
                    hT = ep.tile([64, 512], F32, tag="hT")
                    nc.scalar.activation(out=hT[0:32, :], in_=psH[0:32, :],
                                         func=AF.Identity, bias=bnum_s[:, 0:1])
                    nc.scalar.activation(out=hT[32:64, :], in_=psH[32:64, :],
                                         func=AF.Identity, bias=bcat_s[:, 0:1])
                    hT2 = ep.tile([64, 512], F32, tag="hT2")
                    nc.vector.scalar_tensor_tensor(
                        out=hT2[:], in0=hT[:], scalar=SLOPE, in1=hT[:],
                        op0=OP.mult, op1=OP.max)
                    # transposed x for al/ar
                    psX = epp.tile([64, 512], F32, tag="psX")
                    nc.tensor.matmul(out=psX[:], lhsT=wtog_s[:], rhs=hT2[:],
                                     start=True, stop=True)
                    xT = ep.tile([64, 512], F32, tag="xT")
                    nc.scalar.activation(out=xT[:], in_=psX[:],
                                         func=AF.Identity, bias=btog_s[:, 0:1])
                    xTl = ep.tile([64, 512], F32, tag="xTl")
                    nc.vector.scalar_tensor_tensor(
                        out=xTl[:], in0=xT[:], scalar=SLOPE, in1=xT[:],
                        op0=OP.mult, op1=OP.max)
                    for j in range(4):
                        b = i * 4 + j
                        js = slice(j * 128, (j + 1) * 128)
                        # row-major x block
                        xps = epp.tile([128, 64], F32, tag="xps")
                        nc.tensor.matmul(out=xps[:], lhsT=hT2[:, js],
                                         rhs=wtog_s[:], start=True, stop=True)
                        xb = ep.tile([128, 64], F32, tag="xb")
                        nc.vector.tensor_tensor(out=xb[:], in0=xps[:],
                                                in1=btog_b[:], op=OP.add)
                        xs = ep.tile([128, 64], F32, tag="xs")
                        nc.vector.scalar_tensor_tensor(
                            out=xs[:], in0=xb[:], scalar=SLOPE, in1=xb[:],
                            op0=OP.mult, op1=OP.max)
                        nc.sync.dma_start(
                            out=x0loc[b * 128:(b + 1) * 128, :], in_=xs[:])
                        # al/ar
                        aps = epp.tile([128, 2], F32, tag="aps")
                        nc.tensor.matmul(out=aps[:], lhsT=xTl[:, js],
                                         rhs=attp_s[:], start=True, stop=True)
                        asb = ep.tile([128, 2], F32, tag="asb")
                        nc.scalar.copy(out=asb[:], in_=aps[:])
                        # extended table row block
                        ext = ep.tile([128, ROWB], F32, tag="ext")
                        nc.vector.tensor_tensor(
                            out=ext[:, 0:64], in0=xs[:],
                            in1=_v(dinv_s[:], [[0, 64]], off=b), op=OP.mult)
                        nc.scalar.copy(out=ext[:, 64:65], in_=asb[:, 0:1])
                        nc.vector.memset(ext[:, 65:68], 0.0)
                        nc.sync.dma_start(
                            out=shard0[b * 128:(b + 1) * 128, :], in_=ext[:])
                        nc.sync.dma_start(
                            out=_dap(arr0, b * 128, [[1, 128], [1, 1]]),
                            in_=asb[:, 1:2])

            nc.gpsimd.collective_compute(
                "AllGather", OP.bypass, replica_groups=[list(range(C))],
                ins=[shard0[:, :].opt()], outs=[xg0[:, :].opt()])

            # ---------------- FAConv layers ----------------
            def fa_layer(lp, lpp, xg, arr, out_layer):
                for b in range(NB):
                    K = kb_e[b]
                    it = lp.tile([128, KE], I32, tag="it")
                    nc.sync.dma_start(out=it[:, 0:K],
                                      in_=_dap(idx_e, b * 128 * KE,
                                               [[KE, 128], [1, K]]))
                    mt = lp.tile([128, KE], F32, tag="mt")
                    nc.sync.dma_start(out=mt[:, 0:K],
                                      in_=_dap(mf_e, b * 128 * KE,
                                               [[KE, 128], [1, K]]))
                    g = lp.tile([128, KE, ROWB], F32, tag="g")
                    for k in range(K):
                        nc.gpsimd.indirect_dma_start(
                            out=g[:, k, :], out_offset=None, in_=xg[:, :],
                            in_offset=bass.IndirectOffsetOnAxis(
                                ap=it[:, k:k + 1], axis=0))
                    # ar of this dst block, replicated on all partitions
                    artb = lp.tile([128, 128], F32, tag="artb")
                    nc.sync.dma_start(
                        out=artb[:],
                        in_=_dap(arr, b * 128, [[0, 128], [1, 128]]))
                    # alpha = tanh(al_src + ar_dst)
                    t1 = lp.tile([128, KE, 128], F32, tag="t1")
                    nc.vector.tensor_tensor(
                        out=t1[:, 0:K, :],
                        in0=_v(g[:], [[ROWB, K], [0, 128]], off=64),
                        in1=_v(artb[:], [[0, K], [1, 128]]),
                        op=OP.add)
                    nc.scalar.activation(
                        out=_v(t1[:], [[1, K * 128]]),
                        in_=_v(t1[:], [[1, K * 128]]), func=AF.Tanh)
                    m01 = lp.tile([128, KE, 128], F32, tag="m01")
                    nc.vector.tensor_tensor(
                        out=m01[:, 0:K, :],
                        in0=_v(mt[:], [[1, K], [0, 128]]),
                        in1=_v(iota_s[:], [[0, K], [1, 128]]),
                        op=OP.is_equal)
                    la = lp.tile([128, KE, 128], F32, tag="la")
                    nc.vector.tensor_tensor(
                        out=la[:, 0:K, :], in0=t1[:, 0:K, :],
                        in1=m01[:, 0:K, :], op=OP.mult)
                    agg = lpp.tile([128, 64], F32, tag="agg")
                    for k in range(K):
                        nc.tensor.matmul(out=agg[:], lhsT=la[:, k, :],
                                         rhs=g[:, k, 0:64],
                                         start=(k == 0), stop=(k == K - 1))
                    x0b = lp.tile([128, 64], F32, tag="x0b")
                    nc.sync.dma_start(out=x0b[:],
                                      in_=x0loc[b * 128:(b + 1) * 128, :])
                    xx = lp.tile([128, 64], F32, tag="xx")
                    nc.vector.tensor_tensor(
                        out=xx[:], in0=agg[:],
                        in1=_v(dinv_s[:], [[0, 64]], off=b), op=OP.mult)
                    x1 = lp.tile([128, 64], F32, tag="x1")
                    nc.vector.scalar_tensor_tensor(
                        out=x1[:], in0=x0b[:], scalar=EPS, in1=xx[:],
                        op0=OP.mult, op1=OP.add)
                    if out_layer == 1:
                        x1t_ps = lpp.tile([64, 128], F32, tag="x1t")
                        nc.tensor.transpose(out=x1t_ps[:], in_=x1[:],
                                            identity=ident_s[:])
                        x1t = lp.tile([64, 128], F32, tag="x1ts")
                        nc.scalar.copy(out=x1t[:], in_=x1t_ps[:])
                        aps = lpp.tile([128, 2], F32, tag="aps1")
                        nc.tensor.matmul(out=aps[:], lhsT=x1t[:],
                                         rhs=attp_s[:], start=True, stop=True)
                        asb = lp.tile([128, 2], F32, tag="asb1")
                        nc.scalar.copy(out=asb[:], in_=aps[:])
                        ext = lp.tile([128, ROWB], F32, tag="ext1")
                        nc.vector.tensor_tensor(
                            out=ext[:, 0:64], in0=x1[:],
                            in1=_v(dinv_s[:], [[0, 64]], off=b), op=OP.mult)
                        nc.scalar.copy(out=ext[:, 64:65], in_=asb[:, 0:1])
                        nc.vector.memset(ext[:, 65:68], 0.0)
                        nc.sync.dma_start(
                            out=shard1[b * 128:(b + 1) * 128, :], in_=ext[:])
                        nc.sync.dma_start(
                            out=_dap(arr1, b * 128, [[1, 128], [1, 1]]),
                            in_=asb[:, 1:2])
                    else:
                        sq = lp.tile([128, 64], F32, tag="sq")
                        nc.vector.tensor_tensor(out=sq[:], in0=x1[:],
                                                in1=x1[:], op=OP.mult)
                        x2s = lp.tile([128, 64], F32, tag="x2s")
                        nc.scalar.activation(out=x2s[:], in_=sq[:],
                                             func=AF.Sqrt,
                                             bias=eps_s[:, 0:1])
                        nc.sync.dma_start(
                            out=x2sloc[b * 128:(b + 1) * 128, :], in_=x2s[:])

            with tc.tile_pool(name="lay1", bufs=2) as lp, \
                 tc.tile_pool(name="lay1ps", bufs=2, space="PSUM") as lpp:
                fa_layer(lp, lpp, xg0, arr0, 1)

            nc.gpsimd.collective_compute(
                "AllGather", OP.bypass, replica_groups=[list(range(C))],
                ins=[shard1[:, :].opt()], outs=[xg1[:, :].opt()])

            with tc.tile_pool(name="lay2", bufs=2) as lp, \
                 tc.tile_pool(name="lay2ps", bufs=2, space="PSUM") as lpp:
                fa_layer(lp, lpp, xg1, arr1, 2)

            # ---------------- user segment sums ----------------
            with tc.tile_pool(name="usr", bufs=2) as up, \
                 tc.tile_pool(name="usrps", bufs=2, space="PSUM") as upp:
                for ub in range(UHB):
                    K = kb_u[ub]
                    it = up.tile([128, KU], I32, tag="uit")
                    nc.sync.dma_start(out=it[:, 0:K],
                                      in_=_dap(idx_u, ub * 128 * KU,
                                               [[KU, 128], [1, K]]))
                    mt = up.tile([128, KU], F32, tag="umt")
                    nc.sync.dma_start(out=mt[:, 0:K],
                                      in_=_dap(mf_u, ub * 128 * KU,
                                               [[KU, 128], [1, K]]))
                    g = up.tile([128, KU, 64], F32, tag="ug")
                    for k in range(K):
                        nc.gpsimd.indirect_dma_start(
                            out=g[:, k, :], out_offset=None, in_=x2sloc[:, :],
                            in_offset=bass.IndirectOffsetOnAxis(
                                ap=it[:, k:k + 1], axis=0))
                    m01 = up.tile([128, KU, 128], F32, tag="um01")
                    nc.vector.tensor_tensor(
                        out=m01[:, 0:K, :],
                        in0=_v(mt[:], [[1, K], [0, 128]]),
                        in1=_v(iota_s[:], [[0, K], [1, 128]]),
                        op=OP.is_equal)
                    ups_t = upp.tile([128, 64], F32, tag="ups")
                    for k in range(K):
                        nc.tensor.matmul(out=ups_t[:], lhsT=m01[:, k, :],
                                         rhs=g[:, k, :],
                                         start=(k == 0), stop=(k == K - 1))
                    us = up.tile([128, 64], F32, tag="us")
                    nc.scalar.copy(out=us[:], in_=ups_t[:])
                    nc.sync.dma_start(
                        out=usums[ub * 128:(ub + 1) * 128, :], in_=us[:])

    nc.finalize()
    return nc


_PROG = {}


def _get_program(KE, KU, kb_e, kb_u):
    key = (KE, KU, tuple(int(x) for x in kb_e), tuple(int(x) for x in kb_u))
    if key not in _PROG:
        _PROG[key] = build_program(KE, KU, kb_e, kb_u)
    return _PROG[key]


# --------------------------------------------------------------------------
# Entry point
# --------------------------------------------------------------------------

def run_all(inputs, runner):
    meta = preprocess(inputs)
    KE, KU = meta["KE"], meta["KU"]
    nc = _get_program(KE, KU, meta["kb_e"], meta["kb_u"])

    wnum = np.ascontiguousarray(np.asarray(inputs["W_num"], np.float32))
    wcat = np.ascontiguousarray(np.asarray(inputs["W_cat"], np.float32))
    wtog = np.ascontiguousarray(np.asarray(inputs["W_tog"], np.float32))
    bnum = np.asarray(inputs["b_num"], np.float32).reshape(32, 1).copy()
    bcat = np.asarray(inputs["b_cat"], np.float32).reshape(32, 1).copy()
    btog = np.asarray(inputs["b_tog"], np.float32).reshape(64, 1).copy()
    btogr = np.asarray(inputs["b_tog"], np.float32).reshape(1, 64).copy()
    attp = np.ascontiguousarray(
        np.stack([np.asarray(inputs["att_l"], np.float32),
                  np.asarray(inputs["att_r"], np.float32)], axis=1))
    iota = np.ascontiguousarray(
        np.tile(np.arange(128, dtype=np.float32)[None, :], (128, 1)))
    ident = np.eye(128, dtype=np.float32)

    maps = []
    for c in range(C):
        maps.append({
            "propsT": meta["propsT"][c],
            "dinvt": meta["dinvt"][c],
            "wnum": wnum, "wcat": wcat, "wtog": wtog,
            "bnum": bnum, "bcat": bcat, "btog": btog, "btogr": btogr,
            "attp": attp, "iota": iota, "ident": ident,
            "idx_e": meta["idx_e"][c], "mf_e": meta["mf_e"][c],
            "idx_u": meta["idx_u"][c], "mf_u": meta["mf_u"][c],
        })
    res = runner(nc, maps)

    totals = np.zeros((U, 64), np.float32)
    ulo = meta["ulo"]
    for c in range(C):
        lo = int(ulo[c])
        hi = min(lo + UH, U)
        totals[lo:hi] += res[c]["usums"][:hi - lo]

    re_index = np.asarray(inputs["re_index"]).astype(np.int64, copy=False)
    x3 = totals[re_index]
    wf1 = np.asarray(inputs["W_f1"], np.float32)
    bf1 = np.asarray(inputs["b_f1"], np.float32)
    wlab = np.asarray(inputs["W_lab"], np.float32)
    blab = np.asarray(inputs["b_lab"], np.float32)
    h = x3 @ wf1 + bf1
    h = np.where(h > 0, h, np.float32(SLOPE) * h)
    return (h @ wlab + blab).astype(np.float32)


def kernel(**inputs):
    from concourse.bass_utils import run_bass_kernel_spmd

    def runner(nc, in_maps):
        return run_bass_kernel_spmd(nc, in_maps,
                                    core_ids=list(range(C))).results

    return run_all(inputs, runner)
